# revision 23
# baseline (speedup 1.0000x reference)
"""CCLayer forward on 8 Trainium2 NeuronCores (Bass/Tile).

Math (reference):
    Z  = ZPi[:256];  Pi = ZPi[256:288]          # D=256, p=32, N=262144
    s  = Pi.sum(0)                               # (N,)
    Zo = Z*s - U @ ((U.T@Z - alpha[:,None]) * Pi)
    out = vstack([Zo, Pi])

Kernel formulation (minimizes DVE full-volume work to ONE tensor_tensor
per output element):
    r    = 1/s
    Gn   = (U.T@Z - alpha) * Pi * r              # (32, N)
    T    = Z - U@Gn          # computed fully in PE: identity-matmul
                             # passthrough accumulated with (-U) @ Gn
    Zo   = T * s             # single DVE tensor_tensor vs broadcast of s
Broadcasts of s (and packed r) across partitions are produced by ones-
block matmuls on the PE.  All matmuls run in float32r (11-bit mantissa,
1 cycle/row) — validated rel-err ~1e-4, well inside the grading gate.

Sharding: data-parallel over N; each core gets ZPi[:, i*32768:(i+1)*32768];
U/alpha-derived constants replicated. No communication.
"""

import numpy as np

D, P, N = 256, 32, 262144
NCORES = 8
NC = N // NCORES            # 32768 columns per core
SUB = 512                   # psum-bank tile
PACK = 4                    # subtiles packed per group (32*4 = 128 partitions)
GROUP = SUB * PACK          # 2048
NG = NC // GROUP            # 16 groups

_CACHE = {}


def _build(repeat=1):
    import concourse.bacc as bacc
    import concourse.tile as tile
    import concourse.mybir as mybir

    dt = mybir.dt
    f32, f32r = dt.float32, dt.float32r
    mult = mybir.AluOpType.mult

    nc = bacc.Bacc("TRN2", target_bir_lowering=False, debug=False)

    zpi = nc.dram_tensor("zpi", [D + P, NC], f32r, kind="ExternalInput").ap()
    ublk = nc.dram_tensor("ublk", [128, 64], f32r, kind="ExternalInput").ap()
    negut = nc.dram_tensor("negut", [128, 256], f32r, kind="ExternalInput").ap()
    iden = nc.dram_tensor("iden", [128, 128], f32r, kind="ExternalInput").ap()
    bdiag = nc.dram_tensor("bdiag", [128, 128], f32r, kind="ExternalInput").ap()
    bcols = nc.dram_tensor("bcols", [128, 512], f32r, kind="ExternalInput").ap()
    alneg = nc.dram_tensor("alneg", [128, 1], f32, kind="ExternalInput").ap()
    out = nc.dram_tensor("out", [D + P, NC], f32, kind="ExternalOutput").ap()

    with tile.TileContext(nc) as tc:
        with (
            tc.sbuf_pool(name="wp", bufs=1) as wp,
            tc.sbuf_pool(name="zp", bufs=4) as zp,
            tc.sbuf_pool(name="gp", bufs=3) as gp,
            tc.sbuf_pool(name="bp", bufs=4) as bp,
            tc.sbuf_pool(name="op", bufs=3) as op,
            tc.psum_pool(name="pm", bufs=1) as pm,
            tc.psum_pool(name="pb", bufs=1) as pb,
            tc.psum_pool(name="pt", bufs=3) as pt,
        ):
            # constants (resident)
            u_t = wp.tile([128, 64], f32r)
            nut_t = wp.tile([128, 256], f32r)
            id_t = wp.tile([128, 128], f32r)
            bd_t = wp.tile([128, 128], f32r)
            bc_t = wp.tile([128, 512], f32r)
            al_t = wp.tile([128, 1], f32)
            nc.sync.dma_start(u_t, ublk)
            nc.sync.dma_start(nut_t, negut)
            nc.sync.dma_start(id_t, iden)
            nc.sync.dma_start(bd_t, bdiag)
            nc.sync.dma_start(bc_t, bcols)
            nc.sync.dma_start(al_t, alneg)

            # Pi passthrough rows 256:288 — DRAM->DRAM, exact copy
            nc.gpsimd.dma_start(out[D:, :], zpi[D:, :].bitcast(f32))

            for rep_g in range(repeat * NG):
                g = rep_g % NG
                off = g * GROUP
                z0 = zp.tile([128, GROUP], f32r, tag="z0")
                z1 = zp.tile([128, GROUP], f32r, tag="z1")
                pi4 = zp.tile([128, SUB], f32r, tag="pi4")
                nc.sync.dma_start(z0, zpi[0:128, off : off + GROUP])
                nc.sync.dma_start(z1, zpi[128:256, off : off + GROUP])
                nc.sync.dma_start(
                    pi4,
                    zpi[D:, off : off + GROUP].rearrange("p (j n) -> j p n", j=PACK),
                )

                # group-packed broadcast sums -> reciprocal (psum slot
                # shared with the B broadcasts' pool)
                sb4_ps = pb.tile([128, SUB], f32, tag="b", name="sb4_ps")
                nc.tensor.matmul(sb4_ps, bd_t, pi4, start=True, stop=True)
                rb4 = gp.tile([128, SUB], f32, tag="rb4")
                nc.vector.reciprocal_approx_fast(rb4, sb4_ps)

                # U.T @ Z per subtile into a [32, 4*SUB] psum strip (fp32r
                # matmuls only support dst partition 0 -> no col-tiling).
                # ScalarE evacuates (+ alpha-subtract via bias), gpsimd SWDGE
                # repacks to the partition-packed [128, SUB] layout.
                uz_ps = pm.tile([32, GROUP], f32, tag="uz")
                for j in range(PACK):
                    cs = slice(SUB * j, SUB * (j + 1))
                    nc.tensor.matmul(
                        uz_ps[:, cs], u_t[:, 0:32], z0[:, cs], start=True, stop=False
                    )
                    nc.tensor.matmul(
                        uz_ps[:, cs], u_t[:, 32:64], z1[:, cs], start=False, stop=True
                    )
                uza = gp.tile([32, GROUP], f32, tag="uza")
                uzp = gp.tile([128, SUB], f32, tag="uzp")
                nc.scalar.activation(
                    uza, uz_ps,
                    mybir.ActivationFunctionType.Identity, bias=al_t[0:32, :],
                )
                for j in range(PACK):
                    cs = slice(SUB * j, SUB * (j + 1))
                    nc.gpsimd.dma_start(uzp[32 * j : 32 * (j + 1), :], uza[:, cs])

                # B_j broadcasts (off the critical path: only needs pi4)
                b_sbs = []
                for j in range(PACK):
                    b_ps = pb.tile([128, SUB], f32, tag="b")
                    nc.tensor.matmul(
                        b_ps, bc_t[:, 128 * j : 128 * (j + 1)], pi4,
                        start=True, stop=True,
                    )
                    b_sb = bp.tile([128, SUB], f32, tag="bsb")
                    nc.scalar.copy(b_sb, b_ps)
                    b_sbs.append(b_sb)

                # Gn = (UtZ - alpha) * Pi * r   (packed [128, SUB])
                g1 = gp.tile([128, SUB], f32, tag="g1")
                nc.vector.tensor_tensor(g1, uzp, pi4.bitcast(f32), op=mult)
                g2 = gp.tile([128, SUB], f32r, tag="g2")
                nc.vector.tensor_tensor(g2, g1, rb4, op=mult)

                out0 = op.tile([128, GROUP], f32, tag="o0")
                out1 = op.tile([128, GROUP], f32, tag="o1")
                # per-subtile stores interleaved right after their tt2s:
                # short sem-waits avoid head-of-line blocking; split across
                # SP (h=0) and ACT (h=1) queues
                for j in range(PACK):
                    cs = slice(SUB * j, SUB * (j + 1))
                    js = slice(32 * j, 32 * (j + 1))
                    od = slice(off + SUB * j, off + SUB * (j + 1))
                    for h, (zt, ot) in enumerate(((z0, out0), (z1, out1))):
                        t_ps = pt.tile([128, SUB], f32, tag="t")
                        nc.tensor.matmul(t_ps, id_t, zt[:, cs], start=True, stop=False)
                        nc.tensor.matmul(
                            t_ps,
                            nut_t[js, 128 * h : 128 * (h + 1)],
                            g2[js, :],
                            start=False, stop=True, tile_position=(32 * j, 0),
                        )
                        nc.vector.tensor_tensor(ot[:, cs], t_ps, b_sbs[j], op=mult)
                    nc.sync.dma_start(out[0:128, od], out0[:, cs])
                    nc.scalar.dma_start(out[128:256, od], out1[:, cs])

    nc.compile()
    return nc


def _consts(U, alpha):
    U = np.asarray(U, dtype=np.float32)
    alpha = np.asarray(alpha, dtype=np.float32)
    ublk = np.empty((128, 64), np.float32)
    ublk[:, 0:32] = U[0:128]
    ublk[:, 32:64] = U[128:256]
    negut = np.empty((128, 256), np.float32)
    for h in range(2):
        blkT = -U[128 * h : 128 * (h + 1)].T  # (32, 128)
        for j in range(4):
            negut[32 * j : 32 * (j + 1), 128 * h : 128 * (h + 1)] = blkT
    iden = np.eye(128, dtype=np.float32)
    bdiag = np.zeros((128, 128), np.float32)
    bcols = np.zeros((128, 512), np.float32)
    for j in range(4):
        bdiag[32 * j : 32 * (j + 1), 32 * j : 32 * (j + 1)] = 1.0
        bcols[32 * j : 32 * (j + 1), 128 * j : 128 * (j + 1)] = 1.0
    alneg = (-np.tile(alpha, 4)).astype(np.float32)[:, None].copy()
    return {
        "ublk": ublk, "negut": negut, "iden": iden,
        "bdiag": bdiag, "bcols": bcols, "alneg": alneg,
    }


def _run(inputs, trace=False, trace_kwargs=None):
    from concourse import bass_utils

    if "nc" not in _CACHE:
        _CACHE["nc"] = _build()
    nc = _CACHE["nc"]

    ZPi = np.asarray(inputs["ZPi"], dtype=np.float32)
    consts = _consts(inputs["U"], inputs["alpha"])
    in_maps = []
    for i in range(NCORES):
        m = dict(consts)
        m["zpi"] = np.ascontiguousarray(ZPi[:, i * NC : (i + 1) * NC])
        in_maps.append(m)

    kw = {}
    if trace:
        kw = dict(trace=True, **(trace_kwargs or {}))
    res = bass_utils.run_bass_kernel_spmd(nc, in_maps, core_ids=list(range(NCORES)), **kw)
    full = np.concatenate([res.results[i]["out"] for i in range(NCORES)], axis=1)
    return full, res


def kernel(**inputs) -> np.ndarray:
    out, _ = _run(inputs)
    return out


# revision 28
# speedup vs baseline: 6.6403x; 6.6403x over previous
"""CCLayer forward on 8 Trainium2 NeuronCores (Bass/Tile).

Math (reference):
    Z  = ZPi[:256];  Pi = ZPi[256:288]          # D=256, p=32, N=262144
    s  = Pi.sum(0)                               # (N,)
    Zo = Z*s - U @ ((U.T@Z - alpha[:,None]) * Pi)
    out = vstack([Zo, Pi])

Kernel formulation (minimizes DVE full-volume work to ONE tensor_tensor
per output element):
    r    = 1/s
    Gn   = (U.T@Z - alpha) * Pi * r              # (32, N)
    T    = Z - U@Gn          # computed fully in PE: identity-matmul
                             # passthrough accumulated with (-U) @ Gn
    Zo   = T * s             # single DVE tensor_tensor vs broadcast of s
Broadcasts of s (and packed r) across partitions are produced by ones-
block matmuls on the PE.  All matmuls run in float32r (11-bit mantissa,
1 cycle/row) — validated rel-err ~1e-4, well inside the grading gate.

Sharding: data-parallel over N; each core gets ZPi[:, i*32768:(i+1)*32768];
U/alpha-derived constants replicated. No communication.
"""

import numpy as np

D, P, N = 256, 32, 262144
NCORES = 8
NC = N // NCORES            # 32768 columns per core
SUB = 512                   # psum-bank tile
PACK = 4                    # subtiles packed per group (32*4 = 128 partitions)
GROUP = SUB * PACK          # 2048
NG = NC // GROUP            # 16 groups

_CACHE = {}


def _build(repeat=1):
    import concourse.bacc as bacc
    import concourse.tile as tile
    import concourse.mybir as mybir

    dt = mybir.dt
    f32, f32r = dt.float32, dt.float32r
    mult = mybir.AluOpType.mult

    nc = bacc.Bacc("TRN2", target_bir_lowering=False, debug=False)

    zpi = nc.dram_tensor("zpi", [D + P, NC], f32r, kind="ExternalInput").ap()
    ublk = nc.dram_tensor("ublk", [128, 64], f32r, kind="ExternalInput").ap()
    negut = nc.dram_tensor("negut", [128, 256], f32r, kind="ExternalInput").ap()
    iden = nc.dram_tensor("iden", [128, 128], f32r, kind="ExternalInput").ap()
    bdiag = nc.dram_tensor("bdiag", [128, 128], f32r, kind="ExternalInput").ap()
    bcols = nc.dram_tensor("bcols", [128, 512], f32r, kind="ExternalInput").ap()
    alneg = nc.dram_tensor("alneg", [128, 1], f32, kind="ExternalInput").ap()
    out = nc.dram_tensor("out", [D + P, NC], f32, kind="ExternalOutput").ap()

    with tile.TileContext(nc) as tc:
        with (
            tc.sbuf_pool(name="wp", bufs=1) as wp,
            tc.sbuf_pool(name="zp", bufs=4) as zp,
            tc.sbuf_pool(name="gp", bufs=3) as gp,
            tc.sbuf_pool(name="bp", bufs=4) as bp,
            tc.sbuf_pool(name="op", bufs=3) as op,
            tc.psum_pool(name="pm", bufs=1) as pm,
            tc.psum_pool(name="pb", bufs=1) as pb,
            tc.psum_pool(name="pt", bufs=3) as pt,
        ):
            # constants (resident)
            u_t = wp.tile([128, 64], f32r)
            nut_t = wp.tile([128, 256], f32r)
            id_t = wp.tile([128, 128], f32r)
            bd_t = wp.tile([128, 128], f32r)
            bc_t = wp.tile([128, 512], f32r)
            al_t = wp.tile([128, 1], f32)
            nc.sync.dma_start(u_t, ublk)
            nc.sync.dma_start(nut_t, negut)
            nc.sync.dma_start(id_t, iden)
            nc.sync.dma_start(bd_t, bdiag)
            nc.sync.dma_start(bc_t, bcols)
            nc.sync.dma_start(al_t, alneg)


            for rep_g in range(repeat * NG):
                g = rep_g % NG
                if g == 0:
                    # Pi passthrough rows 256:288 — DRAM->DRAM, exact copy
                    nc.gpsimd.dma_start(out[D:, :], zpi[D:, :].bitcast(f32))
                off = g * GROUP
                z0 = zp.tile([128, GROUP], f32r, tag="z0")
                z1 = zp.tile([128, GROUP], f32r, tag="z1")
                pi4 = zp.tile([128, SUB], f32r, tag="pi4")
                nc.sync.dma_start(z0, zpi[0:128, off : off + GROUP])
                nc.sync.dma_start(z1, zpi[128:256, off : off + GROUP])
                nc.sync.dma_start(
                    pi4,
                    zpi[D:, off : off + GROUP].rearrange("p (j n) -> j p n", j=PACK),
                )

                # group-packed broadcast sums -> reciprocal (psum slot
                # shared with the B broadcasts' pool)
                sb4_ps = pb.tile([128, SUB], f32, tag="b", name="sb4_ps")
                nc.tensor.matmul(sb4_ps, bd_t, pi4, start=True, stop=True)
                rb4 = gp.tile([128, SUB], f32, tag="rb4")
                nc.vector.reciprocal_approx_fast(rb4, sb4_ps)

                # U.T @ Z per subtile into a [32, 4*SUB] psum strip (fp32r
                # matmuls only support dst partition 0 -> no col-tiling).
                # ScalarE evacuates (+ alpha-subtract via bias), gpsimd SWDGE
                # repacks to the partition-packed [128, SUB] layout.
                uz_ps = pm.tile([32, GROUP], f32, tag="uz")
                for j in range(PACK):
                    cs = slice(SUB * j, SUB * (j + 1))
                    nc.tensor.matmul(
                        uz_ps[:, cs], u_t[:, 0:32], z0[:, cs], start=True, stop=False
                    )
                    nc.tensor.matmul(
                        uz_ps[:, cs], u_t[:, 32:64], z1[:, cs], start=False, stop=True
                    )
                uza = gp.tile([32, GROUP], f32, tag="uza")
                uzp = gp.tile([128, SUB], f32, tag="uzp")
                for j in range(PACK):
                    cs = slice(SUB * j, SUB * (j + 1))
                    nc.scalar.activation(
                        uza[:, cs], uz_ps[:, cs],
                        mybir.ActivationFunctionType.Identity, bias=al_t[0:32, :],
                    )
                    nc.gpsimd.dma_start(uzp[32 * j : 32 * (j + 1), :], uza[:, cs])

                # B_j broadcasts (off the critical path: only needs pi4)
                b_sbs = []
                for j in range(PACK):
                    b_ps = pb.tile([128, SUB], f32, tag="b")
                    nc.tensor.matmul(
                        b_ps, bc_t[:, 128 * j : 128 * (j + 1)], pi4,
                        start=True, stop=True,
                    )
                    b_sb = bp.tile([128, SUB], f32, tag="bsb")
                    nc.scalar.copy(b_sb, b_ps)
                    b_sbs.append(b_sb)

                # Gn = (UtZ - alpha) * Pi * r   (packed [128, SUB])
                g1 = gp.tile([128, SUB], f32, tag="g1")
                nc.vector.tensor_tensor(g1, uzp, pi4.bitcast(f32), op=mult)
                g2 = gp.tile([128, SUB], f32r, tag="g2")
                nc.vector.tensor_tensor(g2, g1, rb4, op=mult)

                out0 = op.tile([128, GROUP], f32, tag="o0")
                out1 = op.tile([128, GROUP], f32, tag="o1")
                # per-subtile stores interleaved right after their tt2s:
                # short sem-waits avoid head-of-line blocking; split across
                # SP (h=0) and ACT (h=1) queues
                for j in range(PACK):
                    cs = slice(SUB * j, SUB * (j + 1))
                    js = slice(32 * j, 32 * (j + 1))
                    od = slice(off + SUB * j, off + SUB * (j + 1))
                    for h, (zt, ot) in enumerate(((z0, out0), (z1, out1))):
                        t_ps = pt.tile([128, SUB], f32, tag="t")
                        nc.tensor.matmul(t_ps, id_t, zt[:, cs], start=True, stop=False)
                        nc.tensor.matmul(
                            t_ps,
                            nut_t[js, 128 * h : 128 * (h + 1)],
                            g2[js, :],
                            start=False, stop=True, tile_position=(32 * j, 0),
                        )
                        nc.vector.tensor_tensor(ot[:, cs], t_ps, b_sbs[j], op=mult)
                    nc.sync.dma_start(out[0:128, od], out0[:, cs])
                    nc.scalar.dma_start(out[128:256, od], out1[:, cs])

    nc.compile()
    return nc


def _consts(U, alpha):
    U = np.asarray(U, dtype=np.float32)
    alpha = np.asarray(alpha, dtype=np.float32)
    ublk = np.empty((128, 64), np.float32)
    ublk[:, 0:32] = U[0:128]
    ublk[:, 32:64] = U[128:256]
    negut = np.empty((128, 256), np.float32)
    for h in range(2):
        blkT = -U[128 * h : 128 * (h + 1)].T  # (32, 128)
        for j in range(4):
            negut[32 * j : 32 * (j + 1), 128 * h : 128 * (h + 1)] = blkT
    iden = np.eye(128, dtype=np.float32)
    bdiag = np.zeros((128, 128), np.float32)
    bcols = np.zeros((128, 512), np.float32)
    for j in range(4):
        bdiag[32 * j : 32 * (j + 1), 32 * j : 32 * (j + 1)] = 1.0
        bcols[32 * j : 32 * (j + 1), 128 * j : 128 * (j + 1)] = 1.0
    alneg = (-np.tile(alpha, 4)).astype(np.float32)[:, None].copy()
    return {
        "ublk": ublk, "negut": negut, "iden": iden,
        "bdiag": bdiag, "bcols": bcols, "alneg": alneg,
    }


def _run(inputs, trace=False, trace_kwargs=None):
    from concourse import bass_utils

    if "nc" not in _CACHE:
        _CACHE["nc"] = _build()
    nc = _CACHE["nc"]

    ZPi = np.asarray(inputs["ZPi"], dtype=np.float32)
    consts = _consts(inputs["U"], inputs["alpha"])
    in_maps = []
    for i in range(NCORES):
        m = dict(consts)
        m["zpi"] = np.ascontiguousarray(ZPi[:, i * NC : (i + 1) * NC])
        in_maps.append(m)

    kw = {}
    if trace:
        kw = dict(trace=True, **(trace_kwargs or {}))
    res = bass_utils.run_bass_kernel_spmd(nc, in_maps, core_ids=list(range(NCORES)), **kw)
    full = np.concatenate([res.results[i]["out"] for i in range(NCORES)], axis=1)
    return full, res


def kernel(**inputs) -> np.ndarray:
    out, _ = _run(inputs)
    return out
